# revision 9
# baseline (speedup 1.0000x reference)
"""Trainium2 Bass kernel for fused multi-head attention (16 heads, d_model=1024).

Computes, for x[2, 2048, 1024], w_qkv[3072, 1024], b_qkv[3072]:
    qkv = x @ w_qkv.T + b_qkv
    q, k, v per head (head-interleaved qkv layout)
    out = softmax(q k^T / sqrt(64)) v     reshaped head-major to [2, 2048, 1024]

Sharding: 8 cores = 2 batches x 4 head-groups. Core (b, g) handles batch b,
heads [4g, 4g+4). Everything is local per core; the host slices inputs and
concatenates outputs (the output layout is head-major, so each core's result
is a contiguous slab of the full output).

Per-core dataflow (all on one NeuronCore, fp32 data with fp32r matmuls):
  - host supplies x[b].T (xt, [1024, 2048]) and the core's 768 w_qkv rows,
    transposed and reordered pairwise ([Q(hA)|Q(hB)|K(hA)|K(hB)|V(hA)|V(hB)]
    per head pair), plus the matching bias.
  - projection: psum[feat, s] = wt.T @ xt accumulated over 8 c-tiles; bias
    added during the PSUM->SBUF copy. Q/K stay feature-major ([d, s], the
    layout the scores matmul wants); V is transposed on the PE to [s, d] and
    a ones column is appended (column 64).
  - attention per head: S^T[k, q] = K_t.T @ Q_t (PE, contraction over d=64),
    E = exp(S/8) (ACT, PSUM->SBUF), O[65, q] = [V|1]^T @ E accumulated over
    k-tiles (PE). Row 64 of O is the softmax denominator. O tiles are
    PE-transposed to [q, 65] and each row is scaled by 1/O[q, 64] (DVE).
"""

import os
import sys

import numpy as np

if "/opt/trn_rl_repo" not in sys.path:
    sys.path.insert(0, "/opt/trn_rl_repo")

B = 2
S = 2048
D_MODEL = 1024
NUM_HEADS = 16
HD = 64
N_CORES = 8

P = 128
CT = D_MODEL // P  # 8 contraction tiles for the projection
KT = S // P  # 16 key tiles
SB = 512  # projection s-block (matmul N)
QB = 1024  # attention q-block (one exp instruction)
NQB = S // QB  # 2
PAIRS = 2  # head pairs per core
HPC = 2 * PAIRS  # heads per core
FPC = HPC * 3 * HD  # 768 w rows per core

_CACHE = {}


def _build_program():
    import concourse.bacc as bacc
    import concourse.mybir as mybir
    import concourse.tile as tile
    from concourse.masks import make_identity

    f32 = mybir.dt.float32
    f32r = mybir.dt.float32r
    EXP = mybir.ActivationFunctionType.Exp

    nc = bacc.Bacc("TRN2")

    xt_d = nc.dram_tensor("xt", [D_MODEL, S], f32r, kind="ExternalInput")
    wt_d = nc.dram_tensor("wt", [D_MODEL, FPC], f32r, kind="ExternalInput")
    bias_d = nc.dram_tensor("bias", [PAIRS * 3, P], f32, kind="ExternalInput")
    out_d = nc.dram_tensor("out", [HPC, S, HD], f32, kind="ExternalOutput")

    with tile.TileContext(nc) as tc:
        from contextlib import ExitStack

        with ExitStack() as ctx:
            const = ctx.enter_context(tc.tile_pool(name="const", bufs=1))
            qkp = ctx.enter_context(tc.tile_pool(name="qkp", bufs=1))
            vop = ctx.enter_context(tc.tile_pool(name="vop", bufs=1))
            vtp = ctx.enter_context(tc.tile_pool(name="vtp", bufs=2))
            etp = ctx.enter_context(tc.tile_pool(name="etp", bufs=3))
            osbp = ctx.enter_context(tc.tile_pool(name="osbp", bufs=2))
            ofinp = ctx.enter_context(tc.tile_pool(name="ofinp", bufs=2))
            rcp = ctx.enter_context(tc.tile_pool(name="rcp", bufs=8))
            # PSUM: big = 2-bank tiles (S scores + projection), small = 1-bank
            # (O accumulators + V transposes), tp = 1-bank (output transposes).
            bigp = ctx.enter_context(tc.tile_pool(name="bigp", bufs=2, space="PSUM"))
            smallp = ctx.enter_context(tc.tile_pool(name="smallp", bufs=2, space="PSUM"))
            tpp = ctx.enter_context(tc.tile_pool(name="tpp", bufs=2, space="PSUM"))

            # ---- constant loads ----
            xt_sb = const.tile([P, CT, S], f32r)
            nc.sync.dma_start(out=xt_sb, in_=xt_d.rearrange("(ct p) s -> p ct s", p=P))
            wt_sb = const.tile([P, CT, FPC], f32r)
            nc.sync.dma_start(out=wt_sb, in_=wt_d.rearrange("(ct p) f -> p ct f", p=P))
            bias_sb = const.tile([P, PAIRS * 3], f32)
            nc.sync.dma_start(out=bias_sb, in_=bias_d.rearrange("a b -> b a"))
            ident = const.tile([P, P], f32)
            make_identity(nc, ident)

            # Q/K storage: per pair one [128, 2, 2048] tile; partitions 0:64 =
            # head A, 64:128 = head B; free dim 0 = Q_t, 1 = K_t (both [d, s]).
            qk_sb = []
            for pair in range(PAIRS):
                qk_t = qkp.tile([P, 2, S], f32r, name=f"qk{pair}")
                qk_sb.append(qk_t)
            # V storage: per head [128, 16, 65]: [k-tile partition, k-tile, d+1];
            # column 64 is the ones column (softmax denominator trick).
            ones_sb = const.tile([P, KT], f32)
            nc.vector.memset(ones_sb, 1.0)
            vo_sb = []
            for h in range(HPC):
                vo_t = vop.tile([P, KT, HD + 1], f32r, name=f"vo{h}")
                vo_sb.append(vo_t)
                nc.vector.tensor_copy(vo_t[:, :, HD], ones_sb)

            # ---- projection phase ----
            for pair in range(PAIRS):
                vt_t = vtp.tile([P, S], f32, name="vt", tag="vt")
                for ft in range(3):  # 0 = Q, 1 = K, 2 = V
                    fb = pair * 3 * P + ft * P
                    for sb in range(S // SB):
                        ps = bigp.tile([P, QB], f32, name="proj_ps", tag="big")
                        pslice = ps[:, :SB]
                        for ct in range(CT):
                            nc.tensor.matmul(
                                pslice,
                                lhsT=wt_sb[:, ct, fb : fb + P],
                                rhs=xt_sb[:, ct, sb * SB : (sb + 1) * SB],
                                start=(ct == 0),
                                stop=(ct == CT - 1),
                            )
                        bcol = bias_sb[:, pair * 3 + ft : pair * 3 + ft + 1]
                        if ft < 2:
                            dst = qk_sb[pair][:, ft, sb * SB : (sb + 1) * SB]
                        else:
                            dst = vt_t[:, sb * SB : (sb + 1) * SB]
                        nc.vector.tensor_scalar_add(dst, pslice, bcol)
                # V transpose: [vfeat, s] -> [s, vfeat] in 128x128 tiles.
                for kt in range(KT):
                    tp = smallp.tile([P, P], f32, name="vtp_ps", tag="small")
                    nc.tensor.transpose(tp, vt_t[:, kt * P : (kt + 1) * P], ident)
                    nc.vector.tensor_copy(vo_sb[2 * pair][:, kt, 0:HD], tp[:, 0:HD])
                    nc.vector.tensor_copy(
                        vo_sb[2 * pair + 1][:, kt, 0:HD], tp[:, HD:P]
                    )

            # ---- attention phase ----
            for h in range(HPC):
                pair, half = h // 2, h % 2
                pb = half * HD  # partition base for this head's Q/K
                qk_t = qk_sb[pair]
                vo_t = vo_sb[h]
                for qb in range(NQB):
                    o_ps = [
                        smallp.tile([HD + 1, SB], f32, name=f"o_ps{i}", tag="small")
                        for i in range(QB // SB)
                    ]
                    et_tiles = []
                    # software-pipelined: S(kt) ... exp(kt) on ACT ... O(kt)
                    for kt in range(KT):
                        s_ps = bigp.tile([P, QB], f32, name="s_ps", tag="big")
                        for i in range(QB // SB):
                            nc.tensor.matmul(
                                s_ps[:, i * SB : (i + 1) * SB],
                                lhsT=qk_t[
                                    pb : pb + HD, 1, kt * P : (kt + 1) * P
                                ],
                                rhs=qk_t[
                                    pb : pb + HD, 0, qb * QB + i * SB : qb * QB + (i + 1) * SB
                                ],
                                start=True,
                                stop=True,
                            )
                        et = etp.tile([P, QB], f32r, name="et", tag="et")
                        nc.scalar.activation(et, s_ps, EXP, scale=0.125)
                        et_tiles.append(et)
                        for i in range(QB // SB):
                            nc.tensor.matmul(
                                o_ps[i],
                                lhsT=vo_t[:, kt, :],
                                rhs=et[:, i * SB : (i + 1) * SB],
                                start=(kt == 0),
                                stop=(kt == KT - 1),
                            )
                    # normalize + transpose back to [q, d], then store
                    osb = osbp.tile([HD + 1, QB], f32, name="osb", tag="osb")
                    for i in range(QB // SB):
                        nc.vector.tensor_copy(osb[:, i * SB : (i + 1) * SB], o_ps[i])
                    ofin = ofinp.tile([P, QB // P, HD], f32, name="ofin", tag="ofin")
                    for j in range(QB // P):
                        tp2 = tpp.tile([P, HD + 1], f32, name="ot_ps", tag="tp")
                        nc.tensor.transpose(
                            tp2,
                            osb[:, j * P : (j + 1) * P],
                            ident[: HD + 1, : HD + 1],
                        )
                        rc = rcp.tile([P, 1], f32, name="rc", tag="rc")
                        nc.vector.reciprocal(rc, tp2[:, HD : HD + 1])
                        nc.vector.tensor_scalar_mul(ofin[:, j, :], tp2[:, 0:HD], rc)
                    nc.sync.dma_start(
                        out=out_d[h, qb * QB : (qb + 1) * QB, :].rearrange(
                            "(j p) d -> p j d", p=P
                        ),
                        in_=ofin,
                    )

    nc.finalize()
    return nc


def _get_program():
    if "nc" not in _CACHE:
        _CACHE["nc"] = _build_program()
    return _CACHE["nc"]


def _make_in_maps(x, w_qkv, b_qkv):
    in_maps = []
    for core in range(N_CORES):
        b, g = core // 4, core % 4
        order = []
        for pair in range(PAIRS):
            hA = 4 * g + 2 * pair
            for off in (0, HD, 2 * HD):  # Q, K, V row offsets inside a head
                for h in (hA, hA + 1):
                    order.extend(range(h * 3 * HD + off, h * 3 * HD + off + HD))
        order = np.asarray(order)
        in_maps.append(
            {
                "xt": np.ascontiguousarray(x[b].T),
                "wt": np.ascontiguousarray(w_qkv[order].T),
                "bias": np.ascontiguousarray(b_qkv[order].reshape(PAIRS * 3, P)),
            }
        )
    return in_maps


def _install_ntff_hook():
    """Provide antenv.axon_hooks (absent in this image) so trace=True works."""
    import contextlib
    import ctypes
    import types

    try:
        from antenv.axon_hooks import get_axon_ntff_profile_hook  # noqa: F401

        return
    except ImportError:
        pass
    import antenv

    mod = types.ModuleType("antenv.axon_hooks")
    mod._hook = None

    def set_axon_ntff_profile_hook(h):
        mod._hook = h

    def get_axon_ntff_profile_hook():
        return mod._hook

    mod.set_axon_ntff_profile_hook = set_axon_ntff_profile_hook
    mod.get_axon_ntff_profile_hook = get_axon_ntff_profile_hook
    sys.modules["antenv.axon_hooks"] = mod
    antenv.axon_hooks = mod

    so_path = "/opt/axon/libaxon_pjrt.so"
    if not os.path.exists(so_path):
        return
    lib = ctypes.CDLL(so_path)
    if not hasattr(lib, "axon_start_nrt_profile"):
        return
    lib.axon_start_nrt_profile.argtypes = [
        ctypes.POINTER(ctypes.c_int64),
        ctypes.c_size_t,
    ]
    lib.axon_start_nrt_profile.restype = ctypes.c_int64
    lib.axon_stop_nrt_profile.argtypes = [ctypes.c_char_p]
    lib.axon_stop_nrt_profile.restype = ctypes.c_int64

    @contextlib.contextmanager
    def _hook(output_dir, device_ids):
        import jax

        jax.devices()
        if device_ids:
            ids = (ctypes.c_int64 * len(device_ids))(*device_ids)
            rc = lib.axon_start_nrt_profile(ids, len(device_ids))
        else:
            rc = lib.axon_start_nrt_profile(None, 0)
        if rc != 0:
            raise RuntimeError(f"axon_start_nrt_profile rc={rc}")
        try:
            yield
        finally:
            n = lib.axon_stop_nrt_profile(str(output_dir).encode())
            print(f"profile: {n} file(s) written to {output_dir}")

    set_axon_ntff_profile_hook(_hook)


def kernel(x, w_qkv, b_qkv, trace=False):
    from concourse.bass_utils import run_bass_kernel_spmd

    if trace:
        _install_ntff_hook()

    x = np.ascontiguousarray(np.asarray(x, dtype=np.float32))
    w_qkv = np.ascontiguousarray(np.asarray(w_qkv, dtype=np.float32))
    b_qkv = np.ascontiguousarray(np.asarray(b_qkv, dtype=np.float32))

    nc = _get_program()
    in_maps = _make_in_maps(x, w_qkv, b_qkv)
    res = run_bass_kernel_spmd(nc, in_maps, list(range(N_CORES)), trace=trace)
    _CACHE["last_result"] = res

    out = np.empty((B, S, D_MODEL), dtype=np.float32)
    for core in range(N_CORES):
        b, g = core // 4, core % 4
        out[b].reshape(NUM_HEADS, S, HD)[4 * g : 4 * g + 4] = res.results[core]["out"]
    return out


# revision 10
# speedup vs baseline: 1.5927x; 1.5927x over previous
"""Trainium2 Bass kernel for fused multi-head attention (16 heads, d_model=1024).

Computes, for x[2, 2048, 1024], w_qkv[3072, 1024], b_qkv[3072]:
    qkv = x @ w_qkv.T + b_qkv
    q, k, v per head (head-interleaved qkv layout)
    out = softmax(q k^T / sqrt(64)) v     reshaped head-major to [2, 2048, 1024]

Sharding: 8 cores = 2 batches x 4 head-groups. Core (b, g) handles batch b,
heads [4g, 4g+4). Everything is local per core; the host slices inputs and
concatenates outputs (the output layout is head-major, so each core's result
is a contiguous slab of the full output).

Per-core dataflow (all on one NeuronCore, fp32 data with fp32r matmuls):
  - host supplies x[b].T (xt, [1024, 2048]) and the core's 768 w_qkv rows,
    transposed and reordered pairwise ([Q(hA)|Q(hB)|K(hA)|K(hB)|V(hA)|V(hB)]
    per head pair), plus the matching bias.
  - projection: psum[feat, s] = wt.T @ xt accumulated over 8 c-tiles; bias
    added during the PSUM->SBUF copy. Q/K stay feature-major ([d, s], the
    layout the scores matmul wants); V is transposed on the PE to [s, d] and
    a ones column is appended (column 64).
  - attention per head: S^T[k, q] = K_t.T @ Q_t (PE, contraction over d=64),
    E = exp(S/8) (ACT, PSUM->SBUF), O[65, q] = [V|1]^T @ E accumulated over
    k-tiles (PE). Row 64 of O is the softmax denominator. O tiles are
    PE-transposed to [q, 65] and each row is scaled by 1/O[q, 64] (DVE).
"""

import os
import sys

import numpy as np

if "/opt/trn_rl_repo" not in sys.path:
    sys.path.insert(0, "/opt/trn_rl_repo")

B = 2
S = 2048
D_MODEL = 1024
NUM_HEADS = 16
HD = 64
N_CORES = 8

P = 128
CT = D_MODEL // P  # 8 contraction tiles for the projection
KT = S // P  # 16 key tiles
SB = 512  # projection s-block (matmul N)
QB = 1024  # attention q-block (one exp instruction)
NQB = S // QB  # 2
PAIRS = 2  # head pairs per core
HPC = 2 * PAIRS  # heads per core
FPC = HPC * 3 * HD  # 768 w rows per core

MM_DTYPE = os.environ.get("BASS_MM_DTYPE", "bf16")

_CACHE = {}


def _build_program():
    import concourse.bacc as bacc
    import concourse.mybir as mybir
    import concourse.tile as tile
    from concourse.masks import make_identity

    f32 = mybir.dt.float32
    f32r = mybir.dt.float32r
    bf16 = mybir.dt.bfloat16
    mmdt = bf16 if MM_DTYPE == "bf16" else f32r
    EXP = mybir.ActivationFunctionType.Exp

    nc = bacc.Bacc("TRN2")

    xt_d = nc.dram_tensor("xt", [D_MODEL, S], mmdt, kind="ExternalInput")
    wt_d = nc.dram_tensor("wt", [D_MODEL, FPC], mmdt, kind="ExternalInput")
    bias_d = nc.dram_tensor("bias", [PAIRS * 3, P], f32, kind="ExternalInput")
    out_d = nc.dram_tensor("out", [HPC, S, HD], f32, kind="ExternalOutput")

    with tile.TileContext(nc) as tc:
        from contextlib import ExitStack

        with ExitStack() as ctx:
            const = ctx.enter_context(tc.tile_pool(name="const", bufs=1))
            qkp = ctx.enter_context(tc.tile_pool(name="qkp", bufs=1))
            vop = ctx.enter_context(tc.tile_pool(name="vop", bufs=1))
            vtp = ctx.enter_context(tc.tile_pool(name="vtp", bufs=2))
            etp = ctx.enter_context(tc.tile_pool(name="etp", bufs=3))
            osbp = ctx.enter_context(tc.tile_pool(name="osbp", bufs=2))
            ofinp = ctx.enter_context(tc.tile_pool(name="ofinp", bufs=2))
            rcp = ctx.enter_context(tc.tile_pool(name="rcp", bufs=8))
            # PSUM: big = 2-bank tiles (S scores + projection), small = 1-bank
            # (O accumulators + V transposes), tp = 1-bank (output transposes).
            bigp = ctx.enter_context(tc.tile_pool(name="bigp", bufs=2, space="PSUM"))
            smallp = ctx.enter_context(tc.tile_pool(name="smallp", bufs=2, space="PSUM"))
            tpp = ctx.enter_context(tc.tile_pool(name="tpp", bufs=2, space="PSUM"))

            # ---- constant loads ----
            xt_sb = const.tile([P, CT, S], mmdt)
            nc.sync.dma_start(out=xt_sb, in_=xt_d.rearrange("(ct p) s -> p ct s", p=P))
            wt_sb = const.tile([P, CT, FPC], mmdt)
            nc.sync.dma_start(out=wt_sb, in_=wt_d.rearrange("(ct p) f -> p ct f", p=P))
            bias_sb = const.tile([P, PAIRS * 3], f32)
            nc.sync.dma_start(out=bias_sb, in_=bias_d.rearrange("a b -> b a"))
            ident = const.tile([P, P], f32)
            make_identity(nc, ident)
            ident_mm = const.tile([P, P], mmdt)
            make_identity(nc, ident_mm)

            # Q/K storage: per pair one [128, 2, 2048] tile; partitions 0:64 =
            # head A, 64:128 = head B; free dim 0 = Q_t, 1 = K_t (both [d, s]).
            qk_sb = []
            for pair in range(PAIRS):
                qk_t = qkp.tile([P, 2, S], mmdt, name=f"qk{pair}")
                qk_sb.append(qk_t)
            # V storage: per head [128, 16, 65]: [k-tile partition, k-tile, d+1];
            # column 64 is the ones column (softmax denominator trick).
            ones_sb = const.tile([P, KT], f32)
            nc.vector.memset(ones_sb, 1.0)
            vo_sb = []
            for h in range(HPC):
                vo_t = vop.tile([P, KT, HD + 1], mmdt, name=f"vo{h}")
                vo_sb.append(vo_t)
                nc.vector.tensor_copy(vo_t[:, :, HD], ones_sb)

            # ---- projection phase ----
            for pair in range(PAIRS):
                vt_t = vtp.tile([P, S], mmdt, name="vt", tag="vt")
                for ft in range(3):  # 0 = Q, 1 = K, 2 = V
                    fb = pair * 3 * P + ft * P
                    for sb in range(S // SB):
                        ps = bigp.tile([P, QB], f32, name="proj_ps", tag="big")
                        pslice = ps[:, :SB]
                        for ct in range(CT):
                            nc.tensor.matmul(
                                pslice,
                                lhsT=wt_sb[:, ct, fb : fb + P],
                                rhs=xt_sb[:, ct, sb * SB : (sb + 1) * SB],
                                start=(ct == 0),
                                stop=(ct == CT - 1),
                            )
                        bcol = bias_sb[:, pair * 3 + ft : pair * 3 + ft + 1]
                        if ft < 2:
                            dst = qk_sb[pair][:, ft, sb * SB : (sb + 1) * SB]
                        else:
                            dst = vt_t[:, sb * SB : (sb + 1) * SB]
                        nc.vector.tensor_scalar_add(dst, pslice, bcol)
                # V transpose: [vfeat, s] -> [s, vfeat] in 128x128 tiles.
                for kt in range(KT):
                    tp = smallp.tile([P, P], mmdt, name="vtp_ps", tag="small")
                    nc.tensor.transpose(tp, vt_t[:, kt * P : (kt + 1) * P], ident_mm)
                    nc.vector.tensor_copy(vo_sb[2 * pair][:, kt, 0:HD], tp[:, 0:HD])
                    nc.vector.tensor_copy(
                        vo_sb[2 * pair + 1][:, kt, 0:HD], tp[:, HD:P]
                    )

            # ---- attention phase ----
            for h in range(HPC):
                pair, half = h // 2, h % 2
                pb = half * HD  # partition base for this head's Q/K
                qk_t = qk_sb[pair]
                vo_t = vo_sb[h]
                for qb in range(NQB):
                    o_ps = [
                        smallp.tile([HD + 1, SB], f32, name=f"o_ps{i}", tag="small")
                        for i in range(QB // SB)
                    ]
                    et_tiles = []
                    # software-pipelined: S(kt) ... exp(kt) on ACT ... O(kt)
                    for kt in range(KT):
                        s_ps = bigp.tile([P, QB], f32, name="s_ps", tag="big")
                        for i in range(QB // SB):
                            nc.tensor.matmul(
                                s_ps[:, i * SB : (i + 1) * SB],
                                lhsT=qk_t[
                                    pb : pb + HD, 1, kt * P : (kt + 1) * P
                                ],
                                rhs=qk_t[
                                    pb : pb + HD, 0, qb * QB + i * SB : qb * QB + (i + 1) * SB
                                ],
                                start=True,
                                stop=True,
                            )
                        et = etp.tile([P, QB], mmdt, name="et", tag="et")
                        nc.scalar.activation(et, s_ps, EXP, scale=0.125)
                        et_tiles.append(et)
                        for i in range(QB // SB):
                            nc.tensor.matmul(
                                o_ps[i],
                                lhsT=vo_t[:, kt, :],
                                rhs=et[:, i * SB : (i + 1) * SB],
                                start=(kt == 0),
                                stop=(kt == KT - 1),
                            )
                    # normalize + transpose back to [q, d], then store
                    osb = osbp.tile([HD + 1, QB], f32, name="osb", tag="osb")
                    for i in range(QB // SB):
                        nc.vector.tensor_copy(osb[:, i * SB : (i + 1) * SB], o_ps[i])
                    ofin = ofinp.tile([P, QB // P, HD], f32, name="ofin", tag="ofin")
                    for j in range(QB // P):
                        tp2 = tpp.tile([P, HD + 1], f32, name="ot_ps", tag="tp")
                        nc.tensor.transpose(
                            tp2,
                            osb[:, j * P : (j + 1) * P],
                            ident[: HD + 1, : HD + 1],
                        )
                        rc = rcp.tile([P, 1], f32, name="rc", tag="rc")
                        nc.vector.reciprocal(rc, tp2[:, HD : HD + 1])
                        nc.vector.tensor_scalar_mul(ofin[:, j, :], tp2[:, 0:HD], rc)
                    nc.sync.dma_start(
                        out=out_d[h, qb * QB : (qb + 1) * QB, :].rearrange(
                            "(j p) d -> p j d", p=P
                        ),
                        in_=ofin,
                    )

    nc.finalize()
    return nc


def _get_program():
    if "nc" not in _CACHE:
        _CACHE["nc"] = _build_program()
    return _CACHE["nc"]


def _make_in_maps(x, w_qkv, b_qkv):
    in_maps = []
    for core in range(N_CORES):
        b, g = core // 4, core % 4
        order = []
        for pair in range(PAIRS):
            hA = 4 * g + 2 * pair
            for off in (0, HD, 2 * HD):  # Q, K, V row offsets inside a head
                for h in (hA, hA + 1):
                    order.extend(range(h * 3 * HD + off, h * 3 * HD + off + HD))
        order = np.asarray(order)
        if MM_DTYPE == "bf16":
            import ml_dtypes

            cvt = lambda a: np.ascontiguousarray(a.astype(ml_dtypes.bfloat16))
        else:
            cvt = np.ascontiguousarray
        in_maps.append(
            {
                "xt": cvt(x[b].T),
                "wt": cvt(w_qkv[order].T),
                "bias": np.ascontiguousarray(b_qkv[order].reshape(PAIRS * 3, P)),
            }
        )
    return in_maps


def _install_ntff_hook():
    """Provide antenv.axon_hooks (absent in this image) so trace=True works."""
    import contextlib
    import ctypes
    import types

    try:
        from antenv.axon_hooks import get_axon_ntff_profile_hook  # noqa: F401

        return
    except ImportError:
        pass
    import antenv

    mod = types.ModuleType("antenv.axon_hooks")
    mod._hook = None

    def set_axon_ntff_profile_hook(h):
        mod._hook = h

    def get_axon_ntff_profile_hook():
        return mod._hook

    mod.set_axon_ntff_profile_hook = set_axon_ntff_profile_hook
    mod.get_axon_ntff_profile_hook = get_axon_ntff_profile_hook
    sys.modules["antenv.axon_hooks"] = mod
    antenv.axon_hooks = mod

    so_path = "/opt/axon/libaxon_pjrt.so"
    if not os.path.exists(so_path):
        return
    lib = ctypes.CDLL(so_path)
    if not hasattr(lib, "axon_start_nrt_profile"):
        return
    lib.axon_start_nrt_profile.argtypes = [
        ctypes.POINTER(ctypes.c_int64),
        ctypes.c_size_t,
    ]
    lib.axon_start_nrt_profile.restype = ctypes.c_int64
    lib.axon_stop_nrt_profile.argtypes = [ctypes.c_char_p]
    lib.axon_stop_nrt_profile.restype = ctypes.c_int64

    @contextlib.contextmanager
    def _hook(output_dir, device_ids):
        import jax

        jax.devices()
        if device_ids:
            ids = (ctypes.c_int64 * len(device_ids))(*device_ids)
            rc = lib.axon_start_nrt_profile(ids, len(device_ids))
        else:
            rc = lib.axon_start_nrt_profile(None, 0)
        if rc != 0:
            raise RuntimeError(f"axon_start_nrt_profile rc={rc}")
        try:
            yield
        finally:
            n = lib.axon_stop_nrt_profile(str(output_dir).encode())
            print(f"profile: {n} file(s) written to {output_dir}")

    set_axon_ntff_profile_hook(_hook)


def kernel(x, w_qkv, b_qkv, trace=False):
    from concourse.bass_utils import run_bass_kernel_spmd

    if trace:
        _install_ntff_hook()

    x = np.ascontiguousarray(np.asarray(x, dtype=np.float32))
    w_qkv = np.ascontiguousarray(np.asarray(w_qkv, dtype=np.float32))
    b_qkv = np.ascontiguousarray(np.asarray(b_qkv, dtype=np.float32))

    nc = _get_program()
    in_maps = _make_in_maps(x, w_qkv, b_qkv)
    res = run_bass_kernel_spmd(nc, in_maps, list(range(N_CORES)), trace=trace)
    _CACHE["last_result"] = res

    out = np.empty((B, S, D_MODEL), dtype=np.float32)
    for core in range(N_CORES):
        b, g = core // 4, core % 4
        out[b].reshape(NUM_HEADS, S, HD)[4 * g : 4 * g + 4] = res.results[core]["out"]
    return out
